# revision 19
# baseline (speedup 1.0000x reference)
"""Trainium2 Bass kernel: GQA attention block.

Problem (hardcoded): B=2, T=1024, C=2048, N_HEADS=16, N_KV=4, H=128.
  q = rms_norm(x @ q_kernel); k = rms_norm(x @ k_kernel); v = x @ v_kernel
  q, k: RoPE;  logits = (q/sqrt(H)) @ k^T;  softmax (full, non-causal)
  out = (probs @ v) @ out_kernel

Sharding over 8 cores: core c -> (batch b = c//4, T-slice s = c%4 of 256
query rows).  Each core computes K/V for the full batch (needed by the
non-causal attention) and Q only for its slice; per-core output is the
[256, 2048] slice, gathered on host.

On-chip layout is head-major/transposed: xT [C, T], qT/kT [head_dim, t].
RMS-norm sums-of-squares over the 2048/512-wide rows become ones-matmul
column sums; RoPE is a partition-half swap (tables are host-precomputed,
sign/scale folded).  Softmax skips max-subtraction (logits are rms-normed,
|logit| < 7).  All matmuls run as float32r (fp22 mantissa) at N>=256.
"""
import os
from contextlib import ExitStack

import numpy as np

import concourse.bacc as bacc
import concourse.bass as bass
import concourse.tile as tile
from concourse import mybir
from concourse.bass_utils import run_bass_kernel_spmd

# problem constants
B, T, C = 2, 1024, 2048
N_HEADS, N_KV, H = 16, 4, 128
G = N_HEADS // N_KV      # 4 q heads per kv head
TL = T // 4              # 256 local q rows per core
P = 128                  # partitions
CT = C // P              # 16 contraction tiles
KM = (N_KV * H) // P     # 4 k m-tiles
ST = T // P              # 8 s-tiles
CB = 4                   # out-proj column blocks of 512
F32 = mybir.dt.float32
F32R = mybir.dt.float32r
AF = mybir.ActivationFunctionType
EPS = 1e-6
MAX_TIMESCALE = 10000.0


def _r(ap):
    """float32r view (fp22-truncated matmul read) of an fp32 AP."""
    return ap.bitcast(F32R)


def _f(ap):
    """plain-fp32 view of an f32r AP (for DVE/ACT reads)."""
    return ap.bitcast(F32)


def build_nc():
    nc = bacc.Bacc(None, target_bir_lowering=False)
    t_xT = nc.dram_tensor("xT", [P, CT, T], F32R, kind="ExternalInput")
    t_xTq = nc.dram_tensor("xTq", [P, CT, TL], F32R, kind="ExternalInput")
    t_qw = nc.dram_tensor("qw", [CT, P, CT, P], F32R, kind="ExternalInput")
    t_kw = nc.dram_tensor("kw", [KM, P, CT, P], F32R, kind="ExternalInput")
    t_vw = nc.dram_tensor("vw", [P, CT, N_KV * H], F32R, kind="ExternalInput")
    t_ow = nc.dram_tensor("ow", [CB, P, CT, 512], F32R, kind="ExternalInput")
    t_cq = nc.dram_tensor("cq", [P, TL], F32, kind="ExternalInput")
    t_sq = nc.dram_tensor("sq", [P, TL], F32, kind="ExternalInput")
    t_ck = nc.dram_tensor("ck", [P, T], F32, kind="ExternalInput")
    t_sk = nc.dram_tensor("sk", [P, T], F32, kind="ExternalInput")
    t_sw = nc.dram_tensor("sw", [P, P], F32R, kind="ExternalInput")
    t_on = nc.dram_tensor("on", [P, P], F32R, kind="ExternalInput")
    t_out = nc.dram_tensor("out", [TL, C], F32, kind="ExternalOutput")

    with tile.TileContext(nc) as tc:
        _emit(tc, t_xT, t_xTq, t_qw, t_kw, t_vw, t_ow,
              t_cq, t_sq, t_ck, t_sk, t_sw, t_on, t_out)
    nc.compile()
    return nc


def _rope(nc, dst, src, sw_ps, ctab, stab, tmp):
    """dst = src*ctab + sw_ps*stab.

    sw_ps is swap_halves(src), computed on the PE via a permutation
    matmul (DVE lanes cannot cross partitions); the rotate_half sign
    is folded into the stab table.
    """
    nc.vector.tensor_mul(dst, _f(src), ctab)
    nc.vector.tensor_mul(tmp, sw_ps, stab)
    nc.vector.tensor_add(dst, _f(dst), tmp)


def _emit(tc, t_xT, t_xTq, t_qw, t_kw, t_vw, t_ow, t_cq, t_sq, t_ck, t_sk,
          t_sw, t_on, t_out):
    nc = tc.nc

    with ExitStack() as ctx:
        persist = ctx.enter_context(tc.tile_pool(name="persist", bufs=1))
        qT = persist.tile([P, N_HEADS, TL], F32R)
        kT = persist.tile([P, KM, T], F32R)
        v_sb = persist.tile([P, ST, N_KV * H], F32R)
        ones = persist.tile([P, P], F32R)
        sw = persist.tile([P, P], F32R)
        eps_t = persist.tile([P, 1], F32)
        ckr = persist.tile([P, T], F32)
        skr = persist.tile([P, T], F32)
        cqr = persist.tile([P, TL], F32)
        sqr = persist.tile([P, TL], F32)

        nc.vector.memset(eps_t[:], EPS)
        nc.sync.dma_start(out=ones[:], in_=t_on[:])
        nc.sync.dma_start(out=sw[:], in_=t_sw[:])
        nc.sync.dma_start(out=ckr[:], in_=t_ck[:])
        nc.sync.dma_start(out=skr[:], in_=t_sk[:])
        nc.sync.dma_start(out=cqr[:], in_=t_cq[:])
        nc.sync.dma_start(out=sqr[:], in_=t_sq[:])

        # ---- projection phases share the resident transposed activations --
        with ExitStack() as xctx:
            xtp = xctx.enter_context(tc.tile_pool(name="xt", bufs=1))
            xT = xtp.tile([P, CT, T], F32R)
            xTq = xtp.tile([P, CT, TL], F32R)
            nc.sync.dma_start(out=xT[:], in_=t_xT[:])
            nc.sync.dma_start(out=xTq[:], in_=t_xTq[:])

            # ------------ Phase K: k projection + rms + rope --------------
            with ExitStack() as kctx:
                wkp = kctx.enter_context(tc.tile_pool(name="wk", bufs=2))
                krawp = kctx.enter_context(tc.tile_pool(name="kraw", bufs=1))
                ksqp = kctx.enter_context(tc.tile_pool(name="ksq", bufs=2))
                tmpp = kctx.enter_context(tc.tile_pool(name="ktmp", bufs=1))
                rtmpp = kctx.enter_context(tc.tile_pool(name="rtmp", bufs=2))

                kraw = krawp.tile([P, KM, T], F32R)
                with ExitStack() as pctx:
                    ppk = pctx.enter_context(
                        tc.tile_pool(name="ppk", bufs=2, space="PSUM"))
                    pks = pctx.enter_context(
                        tc.tile_pool(name="pks", bufs=1, space="PSUM"))
                    ksum = pks.tile([P, T], F32)
                    for mt in range(KM):
                        wkt = wkp.tile([P, CT, P], F32R, tag="wk")
                        nc.sync.dma_start(out=wkt[:], in_=t_kw[mt])
                        pk = ppk.tile([P, T], F32, tag="pk")
                        for blk in range(2):
                            o = pk[:, blk * 512:(blk + 1) * 512]
                            for ct in range(CT):
                                nc.tensor.matmul(
                                    o, _r(wkt[:, ct, :]),
                                    _r(xT[:, ct, blk * 512:(blk + 1) * 512]),
                                    start=(ct == 0), stop=(ct == CT - 1))
                        ksq = ksqp.tile([P, T], F32R, tag="ksq")
                        nc.scalar.square(ksq[:], pk[:])
                        nc.vector.tensor_copy(kraw[:, mt, :], pk[:])
                        # accumulate sum of squares over m-tiles
                        for blk in range(2):
                            nc.tensor.matmul(
                                ksum[:, blk * 512:(blk + 1) * 512],
                                _r(ones[:]),
                                _r(ksq[:, blk * 512:(blk + 1) * 512]),
                                start=(mt == 0), stop=(mt == KM - 1))
                    # rstd_k = 1/sqrt(mean + eps); fold into rope tables
                    srt = tmpp.tile([P, T], F32, tag="srt")
                    nc.scalar.activation(srt[:], ksum[:], AF.Sqrt,
                                         bias=eps_t[:],
                                         scale=1.0 / (N_KV * H))
                rstd = tmpp.tile([P, T], F32, tag="rstd")
                nc.vector.reciprocal_approx_fast(out=rstd[:], in_=srt[:])
                nc.vector.tensor_mul(ckr[:], ckr[:], rstd[:])
                nc.vector.tensor_mul(skr[:], skr[:], rstd[:])
                with ExitStack() as pctx:
                    psw = pctx.enter_context(
                        tc.tile_pool(name="psw", bufs=2, space="PSUM"))
                    for mt in range(KM):
                        ksw = psw.tile([P, T], F32, tag="ksw")
                        for blk in range(2):
                            nc.tensor.matmul(
                                ksw[:, blk * 512:(blk + 1) * 512], _r(sw[:]),
                                _r(kraw[:, mt, blk * 512:(blk + 1) * 512]),
                                start=True, stop=True)
                        rtmp = rtmpp.tile([P, T], F32, tag="rtmp")
                        _rope(nc, kT[:, mt, :], kraw[:, mt, :], ksw[:],
                              ckr[:], skr[:], rtmp[:])

            # ------------ Phase V: v projection (natural layout) ----------
            with ExitStack() as vctx:
                vwp = vctx.enter_context(tc.tile_pool(name="vw", bufs=1))
                ppv = vctx.enter_context(
                    tc.tile_pool(name="ppv", bufs=2, space="PSUM"))
                vw = vwp.tile([P, CT, N_KV * H], F32R)
                nc.sync.dma_start(out=vw[:], in_=t_vw[:])
                for tt in range(ST):
                    pv = ppv.tile([P, N_KV * H], F32, tag="pv")
                    for ct in range(CT):
                        nc.tensor.matmul(
                            pv[:], _r(xT[:, ct, tt * P:(tt + 1) * P]),
                            _r(vw[:, ct, :]),
                            start=(ct == 0), stop=(ct == CT - 1))
                    nc.vector.tensor_copy(v_sb[:, tt, :], pv[:])

            # ------------ Phase Q: q projection + rms + rope --------------
            with ExitStack() as qctx:
                wqp = qctx.enter_context(tc.tile_pool(name="wq", bufs=2))
                qrawp = qctx.enter_context(tc.tile_pool(name="qraw", bufs=1))
                qsqp = qctx.enter_context(tc.tile_pool(name="qsq", bufs=2))
                qtmpp = qctx.enter_context(tc.tile_pool(name="qtmp", bufs=1))
                qrtmpp = qctx.enter_context(tc.tile_pool(name="qrtmp", bufs=2))

                qraw = qrawp.tile([P, N_HEADS, TL], F32R)
                with ExitStack() as pctx:
                    ppq = pctx.enter_context(
                        tc.tile_pool(name="ppq", bufs=2, space="PSUM"))
                    pqs = pctx.enter_context(
                        tc.tile_pool(name="pqs", bufs=1, space="PSUM"))
                    qsum = pqs.tile([P, TL], F32)
                    for mt in range(N_HEADS):
                        wqt = wqp.tile([P, CT, P], F32R, tag="wq")
                        nc.sync.dma_start(out=wqt[:], in_=t_qw[mt])
                        pq = ppq.tile([P, TL], F32, tag="pq")
                        for ct in range(CT):
                            nc.tensor.matmul(pq[:], _r(wqt[:, ct, :]),
                                             _r(xTq[:, ct, :]),
                                             start=(ct == 0),
                                             stop=(ct == CT - 1))
                        qsq = qsqp.tile([P, TL], F32R, tag="qsq")
                        nc.scalar.square(qsq[:], pq[:])
                        nc.vector.tensor_copy(qraw[:, mt, :], pq[:])
                        nc.tensor.matmul(qsum[:], _r(ones[:]), _r(qsq[:]),
                                         start=(mt == 0),
                                         stop=(mt == N_HEADS - 1))
                    srtq = qtmpp.tile([P, TL], F32, tag="srtq")
                    nc.scalar.activation(srtq[:], qsum[:], AF.Sqrt,
                                         bias=eps_t[:],
                                         scale=1.0 / (N_HEADS * H))
                rstdq = qtmpp.tile([P, TL], F32, tag="rstdq")
                nc.vector.reciprocal_approx_fast(out=rstdq[:], in_=srtq[:])
                nc.vector.tensor_mul(cqr[:], cqr[:], rstdq[:])
                nc.vector.tensor_mul(sqr[:], sqr[:], rstdq[:])
                with ExitStack() as pctx:
                    pswq = pctx.enter_context(
                        tc.tile_pool(name="pswq", bufs=2, space="PSUM"))
                    for mt in range(N_HEADS):
                        qsw = pswq.tile([P, TL], F32, tag="qsw")
                        nc.tensor.matmul(qsw[:], _r(sw[:]),
                                         _r(qraw[:, mt, :]),
                                         start=True, stop=True)
                        qtmp = qrtmpp.tile([P, TL], F32, tag="qrtmp")
                        _rope(nc, qT[:, mt, :], qraw[:, mt, :], qsw[:],
                              cqr[:], sqr[:], qtmp[:])

        # ---------------- Phase A: attention ------------------------------
        with ExitStack() as actx:
            attnp = actx.enter_context(tc.tile_pool(name="attn", bufs=1))
            encT = attnp.tile([P, N_HEADS, TL], F32R)

            with ExitStack() as kvctx:
                expp = kvctx.enter_context(tc.tile_pool(name="exp", bufs=1))
                rcpp = kvctx.enter_context(tc.tile_pool(name="rcp", bufs=2))
                lp = kvctx.enter_context(
                    tc.tile_pool(name="lp", bufs=2, space="PSUM"))
                sp = kvctx.enter_context(
                    tc.tile_pool(name="sp", bufs=2, space="PSUM"))
                ep = kvctx.enter_context(
                    tc.tile_pool(name="ep", bufs=2, space="PSUM"))

                for kh in range(N_KV):
                    ex = expp.tile([P, ST, G, TL], F32R, tag="ex")
                    for pair in range(2):
                        hlo = 2 * pair
                        q_rhs = qT[:, G * kh + hlo:G * kh + hlo + 2, :]
                        for st2 in range(ST // 2):
                            L = lp.tile([P, 2, 2, TL], F32, tag="L")
                            for j in range(2):
                                st = st2 * 2 + j
                                nc.tensor.matmul(
                                    L[:, j, :, :],
                                    _r(kT[:, kh, st * P:(st + 1) * P]),
                                    _r(q_rhs), start=True, stop=True)
                            nc.scalar.activation(
                                ex[:, st2 * 2:st2 * 2 + 2, hlo:hlo + 2, :],
                                L[:], AF.Exp)
                        # softmax denominators, replicated over partitions
                        S = sp.tile([P, 2, TL], F32, tag="S")
                        for st in range(ST):
                            nc.tensor.matmul(
                                S[:], _r(ones[:]),
                                _r(ex[:, st, hlo:hlo + 2, :]),
                                start=(st == 0), stop=(st == ST - 1))
                        # probs @ v  (unnormalized)
                        E = ep.tile([P, 2, TL], F32, tag="E")
                        for st in range(ST):
                            nc.tensor.matmul(
                                E[:], _r(v_sb[:, st, kh * H:(kh + 1) * H]),
                                _r(ex[:, st, hlo:hlo + 2, :]),
                                start=(st == 0), stop=(st == ST - 1))
                        # normalize while draining PSUM -> SBUF
                        rcp = rcpp.tile([P, 2, TL], F32, tag="rcp")
                        nc.vector.reciprocal_approx_fast(out=rcp[:],
                                                         in_=S[:])
                        nc.vector.tensor_mul(
                            encT[:, G * kh + hlo:G * kh + hlo + 2, :],
                            E[:], rcp[:])

            # ---------------- Phase O: output projection ------------------
            with ExitStack() as octx:
                owp = octx.enter_context(tc.tile_pool(name="ow", bufs=2))
                otp = octx.enter_context(tc.tile_pool(name="ot", bufs=3))
                pop = octx.enter_context(
                    tc.tile_pool(name="po", bufs=2, space="PSUM"))
                for cb in range(CB):
                    owt = owp.tile([P, CT, 512], F32R, tag="ow")
                    nc.sync.dma_start(out=owt[:], in_=t_ow[cb])
                    for tt in range(TL // P):
                        PO = pop.tile([P, 512], F32, tag="PO")
                        for mt in range(CT):
                            nc.tensor.matmul(
                                PO[:], _r(encT[:, mt, tt * P:(tt + 1) * P]),
                                _r(owt[:, mt, :]),
                                start=(mt == 0), stop=(mt == CT - 1))
                        o = otp.tile([P, 512], F32, tag="o")
                        nc.vector.tensor_copy(o[:], PO[:])
                        nc.sync.dma_start(
                            out=t_out[tt * P:(tt + 1) * P,
                                      cb * 512:(cb + 1) * 512],
                            in_=o[:])


# ---------------------------------------------------------------------------
# host side: input prep, sharding, gather
# ---------------------------------------------------------------------------

def _tables():
    fraction = np.arange(0, H, 2, dtype=np.float32) / np.float32(H)
    inv_freq = (1.0 / (MAX_TIMESCALE ** fraction)).astype(np.float32)
    sinusoid = np.arange(T, dtype=np.float32)[:, None] * inv_freq[None, :]
    sinusoid = np.concatenate([sinusoid, sinusoid], axis=-1)  # [T, H]
    sinT = np.sin(sinusoid).T.astype(np.float32)              # [H, T]
    cosT = np.cos(sinusoid).T.astype(np.float32)
    sin_signed = np.concatenate([-sinT[:H // 2], sinT[H // 2:]], axis=0)
    scale = np.float32(1.0) / np.sqrt(np.float32(H)).astype(np.float32)
    return (cosT.copy(), sin_signed.copy(),
            (cosT * scale).astype(np.float32),
            (sin_signed * scale).astype(np.float32))


def _install_trace_shim():
    """Dev-only (KERNEL_TRACE=1): register the NTFF profile hook that this
    agent image's antenv lacks, and skip the artifact cloud upload."""
    import sys
    import types
    try:
        from antenv import axon_hooks  # noqa: F401
        ok = True
    except ImportError:
        try:
            from trn_agent_boot.trn_boot import _ntff_profile_via_ctypes
            hook = _ntff_profile_via_ctypes("/opt/axon/libaxon_pjrt.so")
            m = types.ModuleType("antenv.axon_hooks")
            m.get_axon_ntff_profile_hook = lambda: hook
            m.set_axon_ntff_profile_hook = lambda h: None
            sys.modules["antenv.axon_hooks"] = m
            ok = True
        except Exception as e:  # profiling unavailable; still run
            print(f"trace shim failed: {e!r}")
            ok = False
    if ok:
        import concourse.bass_utils as bu
        bu.upload_artifacts = lambda tmpdir: tmpdir
    return ok


def kernel(x, q_kernel, k_kernel, v_kernel, out_kernel):
    x = np.ascontiguousarray(np.asarray(x, dtype=np.float32))
    qk = np.asarray(q_kernel, dtype=np.float32)
    kk = np.asarray(k_kernel, dtype=np.float32)
    vk = np.asarray(v_kernel, dtype=np.float32)
    ok = np.asarray(out_kernel, dtype=np.float32)

    qw = np.ascontiguousarray(qk.reshape(CT, P, CT, P).transpose(2, 1, 0, 3))
    kw = np.ascontiguousarray(kk.reshape(CT, P, KM, P).transpose(2, 1, 0, 3))
    vw = np.ascontiguousarray(vk.reshape(CT, P, N_KV * H).transpose(1, 0, 2))
    ow = np.ascontiguousarray(ok.reshape(CT, P, CB, 512).transpose(2, 1, 0, 3))
    ck_h, sk_h, cq_full, sq_full = _tables()
    sw_h = np.zeros((P, P), np.float32)
    sw_h[(np.arange(P) + P // 2) % P, np.arange(P)] = 1.0
    on_h = np.ones((P, P), np.float32)

    xt = [np.ascontiguousarray(
        x[b].T.reshape(CT, P, T).transpose(1, 0, 2)) for b in range(B)]

    in_maps = []
    for core in range(8):
        b, s = divmod(core, 4)
        t0 = s * TL
        in_maps.append({
            "xT": xt[b],
            "xTq": np.ascontiguousarray(xt[b][:, :, t0:t0 + TL]),
            "qw": qw, "kw": kw, "vw": vw, "ow": ow,
            "cq": np.ascontiguousarray(cq_full[:, t0:t0 + TL]),
            "sq": np.ascontiguousarray(sq_full[:, t0:t0 + TL]),
            "ck": ck_h, "sk": sk_h, "sw": sw_h,
            "on": on_h,
        })

    nc = build_nc()
    trace = bool(os.environ.get("KERNEL_TRACE"))
    kwargs = {}
    if trace:
        trace = _install_trace_shim()
        if trace:
            tdir = os.environ.get("KERNEL_TRACE_DIR")
            if tdir:
                os.makedirs(tdir, exist_ok=True)
                kwargs["tmpdir"] = tdir
    res = run_bass_kernel_spmd(nc, in_maps, core_ids=list(range(8)),
                               trace=trace, **kwargs)
    out = np.zeros((B, T, C), np.float32)
    for core in range(8):
        b, s = divmod(core, 4)
        out[b, s * TL:(s + 1) * TL] = res.results[core]["out"]
    if trace:
        kernel.last_exec_time_ns = res.exec_time_ns
        kernel.last_profile = res.profile_json
    return out


# revision 21
# speedup vs baseline: 1.1152x; 1.1152x over previous
"""Trainium2 Bass kernel: GQA attention block.

Problem (hardcoded): B=2, T=1024, C=2048, N_HEADS=16, N_KV=4, H=128.
  q = rms_norm(x @ q_kernel); k = rms_norm(x @ k_kernel); v = x @ v_kernel
  q, k: RoPE;  logits = (q/sqrt(H)) @ k^T;  softmax (full, non-causal)
  out = (probs @ v) @ out_kernel

Sharding over 8 cores: core c -> (batch b = c//4, T-slice s = c%4 of 256
query rows).  Each core computes K/V for the full batch (the attention is
non-causal over all 1024 keys) and Q only for its slice; the per-core
[256, 2048] output slices are gathered on host.

On-chip layout is head-major/transposed: xT [C, T], qT/kT [head_dim, t].
The host rolls the key/value sequence so each core's 256 query positions
come first (softmax/AV are permutation-invariant in s; the RoPE tables are
rolled to match), which lets Q-projection read the first 256 columns of the
same resident xT used by K/V.  RMS-norm sums-of-squares become ones-matmul
column sums; RoPE's rotate-half is a constant permutation matmul on the PE
(DVE lanes cannot cross partitions), with sign and 1/sqrt(H) folded into
host-precomputed tables.  Softmax skips max-subtraction (logits are
rms-normed; |logit| < 7).  All matmuls run as float32r (fp22 mantissa).
"""
import os
from contextlib import ExitStack

import numpy as np

import concourse.bacc as bacc
import concourse.bass as bass
import concourse.tile as tile
from concourse import mybir
from concourse.bass_utils import run_bass_kernel_spmd

# problem constants
B, T, C = 2, 1024, 2048
N_HEADS, N_KV, H = 16, 4, 128
G = N_HEADS // N_KV      # 4 q heads per kv head
TL = T // 4              # 256 local q rows per core
P = 128                  # partitions
CT = C // P              # 16 contraction tiles
KM = (N_KV * H) // P     # 4 k m-tiles
ST = T // P              # 8 s-tiles
CB = 4                   # out-proj column blocks of 512
F32 = mybir.dt.float32
F32R = mybir.dt.float32r
AF = mybir.ActivationFunctionType
EPS = 1e-6
MAX_TIMESCALE = 10000.0


def _r(ap):
    """float32r view (fp22-truncated matmul read) of an fp32 AP."""
    return ap.bitcast(F32R)


def _f(ap):
    """plain-fp32 view of an f32r AP (for DVE/ACT reads)."""
    return ap.bitcast(F32)


def build_nc():
    nc = bacc.Bacc(None, target_bir_lowering=False)
    t_xT = nc.dram_tensor("xT", [P, CT, T], F32R, kind="ExternalInput")
    t_qw = nc.dram_tensor("qw", [CT, P, CT, P], F32R, kind="ExternalInput")
    t_kw = nc.dram_tensor("kw", [KM, P, CT, P], F32R, kind="ExternalInput")
    t_vw = nc.dram_tensor("vw", [P, CT, N_KV * H], F32R, kind="ExternalInput")
    t_ow = nc.dram_tensor("ow", [CB, P, CT, 512], F32R, kind="ExternalInput")
    t_cq = nc.dram_tensor("cq", [P, TL], F32, kind="ExternalInput")
    t_sq = nc.dram_tensor("sq", [P, TL], F32, kind="ExternalInput")
    t_ck = nc.dram_tensor("ck", [P, T], F32, kind="ExternalInput")
    t_sk = nc.dram_tensor("sk", [P, T], F32, kind="ExternalInput")
    t_sw = nc.dram_tensor("sw", [P, P], F32R, kind="ExternalInput")
    t_on = nc.dram_tensor("on", [P, P], F32R, kind="ExternalInput")
    t_out = nc.dram_tensor("out", [TL, C], F32, kind="ExternalOutput")

    with tile.TileContext(nc) as tc:
        _emit(tc, t_xT, t_qw, t_kw, t_vw, t_ow,
              t_cq, t_sq, t_ck, t_sk, t_sw, t_on, t_out)
    nc.compile()
    return nc


def _rope(nc, dst, src, sw_ps, ctab, stab, tmp):
    """dst = src*ctab + sw_ps*stab.

    sw_ps is swap_halves(src) (PE permutation-matmul result in PSUM);
    the rotate_half sign lives in the stab table.
    """
    nc.vector.tensor_mul(dst, _f(src), ctab)
    nc.vector.tensor_mul(tmp, sw_ps, stab)
    nc.vector.tensor_add(dst, _f(dst), tmp)


def _emit(tc, t_xT, t_qw, t_kw, t_vw, t_ow, t_cq, t_sq, t_ck, t_sk,
          t_sw, t_on, t_out):
    nc = tc.nc

    with ExitStack() as ctx:
        persist = ctx.enter_context(tc.tile_pool(name="persist", bufs=1))
        qT = persist.tile([P, N_HEADS, TL], F32R)
        kT = persist.tile([P, KM, T], F32R)
        v_sb = persist.tile([P, ST, N_KV * H], F32R)
        ones = persist.tile([P, P], F32R)
        nc.sync.dma_start(out=ones[:], in_=t_on[:])

        with ExitStack() as xctx:
            xtp = xctx.enter_context(tc.tile_pool(name="xt", bufs=1))
            xT = xtp.tile([P, CT, T], F32R)
            for ct in range(CT):
                nc.sync.dma_start(out=xT[:, ct, :], in_=t_xT[:, ct, :])

            # -------- K + Q projection / norm / rope (tables scoped) ------
            with ExitStack() as tctx:
                tabp = tctx.enter_context(tc.tile_pool(name="tab", bufs=1))
                sw = tabp.tile([P, P], F32R)
                eps_t = tabp.tile([P, 1], F32)
                ckr = tabp.tile([P, T], F32)
                skr = tabp.tile([P, T], F32)
                cqr = tabp.tile([P, TL], F32)
                sqr = tabp.tile([P, TL], F32)
                cq2 = tabp.tile([P, 2, TL], F32)
                sq2 = tabp.tile([P, 2, TL], F32)
                nc.vector.memset(eps_t[:], EPS)
                nc.sync.dma_start(out=sw[:], in_=t_sw[:])
                nc.sync.dma_start(out=ckr[:], in_=t_ck[:])
                nc.sync.dma_start(out=skr[:], in_=t_sk[:])
                nc.sync.dma_start(out=cqr[:], in_=t_cq[:])
                nc.sync.dma_start(out=sqr[:], in_=t_sq[:])

                # ---------------- Phase K ----------------
                with ExitStack() as kctx:
                    wkp = kctx.enter_context(tc.tile_pool(name="wk", bufs=2))
                    krawp = kctx.enter_context(
                        tc.tile_pool(name="kraw", bufs=1))
                    ksqp = kctx.enter_context(
                        tc.tile_pool(name="ksq", bufs=KM))
                    tmpp = kctx.enter_context(
                        tc.tile_pool(name="ktmp", bufs=1))
                    rtmpp = kctx.enter_context(
                        tc.tile_pool(name="rtmp", bufs=2))

                    kraw = krawp.tile([P, KM, T], F32R)
                    ksqs = []
                    with ExitStack() as pctx:
                        ppk = pctx.enter_context(
                            tc.tile_pool(name="ppk", bufs=2, space="PSUM"))
                        pks = pctx.enter_context(
                            tc.tile_pool(name="pks", bufs=1, space="PSUM"))
                        for mt in range(KM):
                            wkt = wkp.tile([P, CT, P], F32R, tag="wk")
                            nc.sync.dma_start(out=wkt[:], in_=t_kw[mt])
                            pk = ppk.tile([P, T], F32, tag="pk")
                            for blk in range(2):
                                o = pk[:, blk * 512:(blk + 1) * 512]
                                for ct in range(CT):
                                    nc.tensor.matmul(
                                        o, _r(wkt[:, ct, :]),
                                        _r(xT[:, ct,
                                              blk * 512:(blk + 1) * 512]),
                                        start=(ct == 0), stop=(ct == CT - 1))
                            ksq = ksqp.tile([P, T], F32R, tag="ksq")
                            nc.scalar.square(ksq[:], pk[:])
                            nc.vector.tensor_copy(kraw[:, mt, :], pk[:])
                            ksqs.append(ksq)
                        # deferred: sum of squares over m-tiles
                        ksum = pks.tile([P, T], F32)
                        for mt in range(KM):
                            for blk in range(2):
                                nc.tensor.matmul(
                                    ksum[:, blk * 512:(blk + 1) * 512],
                                    _r(ones[:]),
                                    _r(ksqs[mt][:,
                                                blk * 512:(blk + 1) * 512]),
                                    start=(mt == 0), stop=(mt == KM - 1))
                        srt = tmpp.tile([P, T], F32, tag="srt")
                        nc.scalar.activation(srt[:], ksum[:], AF.Sqrt,
                                             bias=eps_t[:],
                                             scale=1.0 / (N_KV * H))
                    rstd = tmpp.tile([P, T], F32, tag="rstd")
                    nc.vector.reciprocal_approx_fast(out=rstd[:], in_=srt[:])
                    nc.vector.tensor_mul(ckr[:], ckr[:], rstd[:])
                    nc.vector.tensor_mul(skr[:], skr[:], rstd[:])
                    with ExitStack() as pctx:
                        psw = pctx.enter_context(
                            tc.tile_pool(name="psw", bufs=KM, space="PSUM"))
                        ksws = []
                        for mt in range(KM):
                            ksw = psw.tile([P, T], F32, tag="ksw")
                            for blk in range(2):
                                nc.tensor.matmul(
                                    ksw[:, blk * 512:(blk + 1) * 512],
                                    _r(sw[:]),
                                    _r(kraw[:, mt,
                                            blk * 512:(blk + 1) * 512]),
                                    start=True, stop=True)
                            ksws.append(ksw)
                        for mt in range(KM):
                            rtmp = rtmpp.tile([P, T], F32, tag="rtmp")
                            _rope(nc, kT[:, mt, :], kraw[:, mt, :],
                                  ksws[mt][:], ckr[:], skr[:], rtmp[:])

                # ---------------- Phase Q ----------------
                with ExitStack() as qctx:
                    wqp = qctx.enter_context(tc.tile_pool(name="wq", bufs=3))
                    qrawp = qctx.enter_context(
                        tc.tile_pool(name="qraw", bufs=1))
                    qsqp = qctx.enter_context(
                        tc.tile_pool(name="qsq", bufs=N_HEADS))
                    qtmpp = qctx.enter_context(
                        tc.tile_pool(name="qtmp", bufs=1))
                    qrtmpp = qctx.enter_context(
                        tc.tile_pool(name="qrtmp", bufs=2))

                    qraw = qrawp.tile([P, N_HEADS, TL], F32R)
                    qsqs = []
                    with ExitStack() as pctx:
                        ppq = pctx.enter_context(
                            tc.tile_pool(name="ppq", bufs=2, space="PSUM"))
                        pqs = pctx.enter_context(
                            tc.tile_pool(name="pqs", bufs=1, space="PSUM"))
                        for mt in range(N_HEADS):
                            wqt = wqp.tile([P, CT, P], F32R, tag="wq")
                            nc.sync.dma_start(out=wqt[:], in_=t_qw[mt])
                            pq = ppq.tile([P, TL], F32, tag="pq")
                            for ct in range(CT):
                                nc.tensor.matmul(
                                    pq[:], _r(wqt[:, ct, :]),
                                    _r(xT[:, ct, 0:TL]),
                                    start=(ct == 0), stop=(ct == CT - 1))
                            qsq = qsqp.tile([P, TL], F32R, tag="qsq")
                            nc.scalar.square(qsq[:], pq[:])
                            nc.vector.tensor_copy(qraw[:, mt, :], pq[:])
                            qsqs.append(qsq)
                        qsum = pqs.tile([P, TL], F32)
                        for mt in range(N_HEADS):
                            nc.tensor.matmul(qsum[:], _r(ones[:]),
                                             _r(qsqs[mt][:]),
                                             start=(mt == 0),
                                             stop=(mt == N_HEADS - 1))
                        srtq = qtmpp.tile([P, TL], F32, tag="srtq")
                        nc.scalar.activation(srtq[:], qsum[:], AF.Sqrt,
                                             bias=eps_t[:],
                                             scale=1.0 / (N_HEADS * H))
                    rstdq = qtmpp.tile([P, TL], F32, tag="rstdq")
                    nc.vector.reciprocal_approx_fast(out=rstdq[:],
                                                     in_=srtq[:])
                    for h in range(2):
                        nc.vector.tensor_mul(cq2[:, h, :], cqr[:], rstdq[:])
                        nc.vector.tensor_mul(sq2[:, h, :], sqr[:], rstdq[:])
                    with ExitStack() as pctx:
                        pswq = pctx.enter_context(
                            tc.tile_pool(name="pswq", bufs=N_HEADS // 2,
                                         space="PSUM"))
                        qsws = []
                        for j in range(N_HEADS // 2):
                            qsw = pswq.tile([P, 2, TL], F32, tag="qsw")
                            for h in range(2):
                                nc.tensor.matmul(qsw[:, h, :], _r(sw[:]),
                                                 _r(qraw[:, 2 * j + h, :]),
                                                 start=True, stop=True)
                            qsws.append(qsw)
                        # rope per pair: full-tile PSUM reads only after
                        # both halves are written (PSUM bank safety)
                        for j in range(N_HEADS // 2):
                            qtmp = qrtmpp.tile([P, 2, TL], F32, tag="qrtmp")
                            _rope(nc, qT[:, 2 * j:2 * j + 2, :],
                                  qraw[:, 2 * j:2 * j + 2, :],
                                  qsws[j][:], cq2[:], sq2[:], qtmp[:])

            # ---------------- Phase V ----------------
            with ExitStack() as vctx:
                vwp = vctx.enter_context(tc.tile_pool(name="vw", bufs=1))
                ppv = vctx.enter_context(
                    tc.tile_pool(name="ppv", bufs=2, space="PSUM"))
                vw = vwp.tile([P, CT, N_KV * H], F32R)
                for c4 in range(4):
                    nc.sync.dma_start(out=vw[:, 4 * c4:4 * c4 + 4, :],
                                      in_=t_vw[:, 4 * c4:4 * c4 + 4, :])
                for tt in range(ST):
                    pv = ppv.tile([P, N_KV * H], F32, tag="pv")
                    for ct in range(CT):
                        nc.tensor.matmul(
                            pv[:], _r(xT[:, ct, tt * P:(tt + 1) * P]),
                            _r(vw[:, ct, :]),
                            start=(ct == 0), stop=(ct == CT - 1))
                    nc.vector.tensor_copy(v_sb[:, tt, :], pv[:])

        # ---------------- Phase A: attention ------------------------------
        with ExitStack() as actx:
            attnp = actx.enter_context(tc.tile_pool(name="attn", bufs=1))
            encT = attnp.tile([P, N_HEADS, TL], F32R)
            owp = actx.enter_context(tc.tile_pool(name="ow", bufs=2))
            # prefetch first two out-proj weight blocks during attention
            owts = []
            for cb in range(2):
                owt = owp.tile([P, CT, 512], F32R, tag="ow")
                nc.sync.dma_start(out=owt[:], in_=t_ow[cb])
                owts.append(owt)

            with ExitStack() as kvctx:
                expp = kvctx.enter_context(tc.tile_pool(name="exp", bufs=1))
                rcpp = kvctx.enter_context(tc.tile_pool(name="rcp", bufs=2))
                lp = kvctx.enter_context(
                    tc.tile_pool(name="lp", bufs=3, space="PSUM"))
                sp = kvctx.enter_context(
                    tc.tile_pool(name="sp", bufs=1, space="PSUM"))
                ep = kvctx.enter_context(
                    tc.tile_pool(name="ep", bufs=1, space="PSUM"))

                for kh in range(N_KV):
                    ex = expp.tile([P, ST, G, TL], F32R, tag="ex")
                    for pair in range(2):
                        hlo = 2 * pair
                        q_rhs = qT[:, G * kh + hlo:G * kh + hlo + 2, :]
                        for st2 in range(ST // 2):
                            L = lp.tile([P, 2, 2, TL], F32, tag="L")
                            for j in range(2):
                                st = st2 * 2 + j
                                nc.tensor.matmul(
                                    L[:, j, :, :],
                                    _r(kT[:, kh, st * P:(st + 1) * P]),
                                    _r(q_rhs), start=True, stop=True)
                            nc.scalar.activation(
                                ex[:, st2 * 2:st2 * 2 + 2, hlo:hlo + 2, :],
                                L[:], AF.Exp)
                        # softmax denominators, replicated over partitions
                        S = sp.tile([P, 2, TL], F32, tag="S")
                        for st in range(ST):
                            nc.tensor.matmul(
                                S[:], _r(ones[:]),
                                _r(ex[:, st, hlo:hlo + 2, :]),
                                start=(st == 0), stop=(st == ST - 1))
                        # probs @ v  (unnormalized)
                        E = ep.tile([P, 2, TL], F32, tag="E")
                        for st in range(ST):
                            nc.tensor.matmul(
                                E[:], _r(v_sb[:, st, kh * H:(kh + 1) * H]),
                                _r(ex[:, st, hlo:hlo + 2, :]),
                                start=(st == 0), stop=(st == ST - 1))
                        # normalize while draining PSUM -> SBUF
                        rcp = rcpp.tile([P, 2, TL], F32, tag="rcp")
                        nc.vector.reciprocal_approx_fast(out=rcp[:],
                                                         in_=S[:])
                        nc.vector.tensor_mul(
                            encT[:, G * kh + hlo:G * kh + hlo + 2, :],
                            E[:], rcp[:])

            # ---------------- Phase O: output projection ------------------
            with ExitStack() as octx:
                otp = octx.enter_context(tc.tile_pool(name="ot", bufs=3))
                pop = octx.enter_context(
                    tc.tile_pool(name="po", bufs=2, space="PSUM"))
                for cb in range(CB):
                    if cb < 2:
                        owt = owts[cb]
                    else:
                        owt = owp.tile([P, CT, 512], F32R, tag="ow")
                        nc.sync.dma_start(out=owt[:], in_=t_ow[cb])
                    for tt in range(TL // P):
                        PO = pop.tile([P, 512], F32, tag="PO")
                        for mt in range(CT):
                            nc.tensor.matmul(
                                PO[:], _r(encT[:, mt, tt * P:(tt + 1) * P]),
                                _r(owt[:, mt, :]),
                                start=(mt == 0), stop=(mt == CT - 1))
                        o = otp.tile([P, 512], F32, tag="o")
                        nc.vector.tensor_copy(o[:], PO[:])
                        nc.sync.dma_start(
                            out=t_out[tt * P:(tt + 1) * P,
                                      cb * 512:(cb + 1) * 512],
                            in_=o[:])


# ---------------------------------------------------------------------------
# host side: input prep, sharding, gather
# ---------------------------------------------------------------------------

def _tables():
    fraction = np.arange(0, H, 2, dtype=np.float32) / np.float32(H)
    inv_freq = (1.0 / (MAX_TIMESCALE ** fraction)).astype(np.float32)
    sinusoid = np.arange(T, dtype=np.float32)[:, None] * inv_freq[None, :]
    sinusoid = np.concatenate([sinusoid, sinusoid], axis=-1)  # [T, H]
    sinT = np.sin(sinusoid).T.astype(np.float32)              # [H, T]
    cosT = np.cos(sinusoid).T.astype(np.float32)
    sin_signed = np.concatenate([-sinT[:H // 2], sinT[H // 2:]], axis=0)
    scale = np.float32(1.0) / np.sqrt(np.float32(H)).astype(np.float32)
    return (cosT.copy(), sin_signed.copy(),
            (cosT * scale).astype(np.float32),
            (sin_signed * scale).astype(np.float32))


def make_in_maps(x, q_kernel, k_kernel, v_kernel, out_kernel):
    x = np.ascontiguousarray(np.asarray(x, dtype=np.float32))
    qk = np.asarray(q_kernel, dtype=np.float32)
    kk = np.asarray(k_kernel, dtype=np.float32)
    vk = np.asarray(v_kernel, dtype=np.float32)
    ok = np.asarray(out_kernel, dtype=np.float32)

    qw = np.ascontiguousarray(qk.reshape(CT, P, CT, P).transpose(2, 1, 0, 3))
    kw = np.ascontiguousarray(kk.reshape(CT, P, KM, P).transpose(2, 1, 0, 3))
    vw = np.ascontiguousarray(vk.reshape(CT, P, N_KV * H).transpose(1, 0, 2))
    ow = np.ascontiguousarray(
        ok.reshape(CT, P, CB, 512).transpose(2, 1, 0, 3))
    ck_h, sk_h, cq_full, sq_full = _tables()
    sw_h = np.zeros((P, P), np.float32)
    sw_h[(np.arange(P) + P // 2) % P, np.arange(P)] = 1.0
    on_h = np.ones((P, P), np.float32)

    xt = [np.ascontiguousarray(
        x[b].T.reshape(CT, P, T).transpose(1, 0, 2)) for b in range(B)]

    in_maps = []
    for core in range(8):
        b, s = divmod(core, 4)
        t0 = s * TL
        # roll the key/value sequence so this core's query block is first;
        # softmax over s is permutation-invariant, RoPE tables roll along.
        in_maps.append({
            "xT": np.ascontiguousarray(np.roll(xt[b], -t0, axis=2)),
            "qw": qw, "kw": kw, "vw": vw, "ow": ow,
            "cq": np.ascontiguousarray(
                np.roll(cq_full, -t0, axis=1)[:, :TL]),
            "sq": np.ascontiguousarray(
                np.roll(sq_full, -t0, axis=1)[:, :TL]),
            "ck": np.ascontiguousarray(np.roll(ck_h, -t0, axis=1)),
            "sk": np.ascontiguousarray(np.roll(sk_h, -t0, axis=1)),
            "sw": sw_h, "on": on_h,
        })
    return in_maps


def _install_trace_shim():
    """Dev-only (KERNEL_TRACE=1): register the NTFF profile hook that this
    agent image's antenv lacks, and skip the artifact cloud upload."""
    import sys
    import types
    try:
        from antenv import axon_hooks  # noqa: F401
        ok = True
    except ImportError:
        try:
            from trn_agent_boot.trn_boot import _ntff_profile_via_ctypes
            hook = _ntff_profile_via_ctypes("/opt/axon/libaxon_pjrt.so")
            m = types.ModuleType("antenv.axon_hooks")
            m.get_axon_ntff_profile_hook = lambda: hook
            m.set_axon_ntff_profile_hook = lambda h: None
            sys.modules["antenv.axon_hooks"] = m
            ok = True
        except Exception as e:  # profiling unavailable; still run
            print(f"trace shim failed: {e!r}")
            ok = False
    if ok:
        import concourse.bass_utils as bu
        bu.upload_artifacts = lambda tmpdir: tmpdir
    return ok


def kernel(x, q_kernel, k_kernel, v_kernel, out_kernel):
    in_maps = make_in_maps(x, q_kernel, k_kernel, v_kernel, out_kernel)
    nc = build_nc()
    trace = bool(os.environ.get("KERNEL_TRACE"))
    kwargs = {}
    if trace:
        trace = _install_trace_shim()
        if trace:
            tdir = os.environ.get("KERNEL_TRACE_DIR")
            if tdir:
                os.makedirs(tdir, exist_ok=True)
                kwargs["tmpdir"] = tdir
    res = run_bass_kernel_spmd(nc, in_maps, core_ids=list(range(8)),
                               trace=trace, **kwargs)
    out = np.zeros((B, T, C), np.float32)
    for core in range(8):
        b, s = divmod(core, 4)
        out[b, s * TL:(s + 1) * TL] = res.results[core]["out"]
    if trace:
        kernel.last_exec_time_ns = res.exec_time_ns
        kernel.last_profile = res.profile_json
    return out


# revision 26
# speedup vs baseline: 1.1271x; 1.0107x over previous
"""Trainium2 Bass kernel: GQA attention block.

Problem (hardcoded): B=2, T=1024, C=2048, N_HEADS=16, N_KV=4, H=128.
  q = rms_norm(x @ q_kernel); k = rms_norm(x @ k_kernel); v = x @ v_kernel
  q, k: RoPE;  logits = (q/sqrt(H)) @ k^T;  softmax (full, non-causal)
  out = (probs @ v) @ out_kernel

Sharding over 8 cores: core c -> (batch b = c//4, T-slice s = c%4 of 256
query rows).  Each core computes K/V for the full batch (the attention is
non-causal over all 1024 keys) and Q only for its slice; the per-core
[256, 2048] output slices are gathered on host.

On-chip layout is head-major/transposed: xT [C, T], qT/kT [head_dim, t].
The host rolls the key/value sequence so each core's 256 query positions
come first (softmax/AV are permutation-invariant in s; the RoPE tables are
rolled to match), which lets Q-projection read the first 256 columns of the
same resident xT used by K/V.  RMS-norm sums-of-squares become ones-matmul
column sums; RoPE's rotate-half is a constant permutation matmul on the PE
(DVE lanes cannot cross partitions), with sign and 1/sqrt(H) folded into
host-precomputed tables.  Softmax skips max-subtraction (logits are
rms-normed; |logit| < 7).  All matmuls run as float32r (fp22 mantissa).
"""
import os
from contextlib import ExitStack

import numpy as np

import concourse.bacc as bacc
import concourse.bass as bass
import concourse.tile as tile
from concourse import mybir
from concourse.bass_utils import run_bass_kernel_spmd

# problem constants
B, T, C = 2, 1024, 2048
N_HEADS, N_KV, H = 16, 4, 128
G = N_HEADS // N_KV      # 4 q heads per kv head
TL = T // 4              # 256 local q rows per core
P = 128                  # partitions
CT = C // P              # 16 contraction tiles
KM = (N_KV * H) // P     # 4 k m-tiles
ST = T // P              # 8 s-tiles
CB = 4                   # out-proj column blocks of 512
F32 = mybir.dt.float32
F32R = mybir.dt.float32r
BF16 = mybir.dt.bfloat16
AF = mybir.ActivationFunctionType
EPS = 1e-6
MAX_TIMESCALE = 10000.0


def _r(ap):
    """float32r view (fp22-truncated matmul read) of an fp32 AP."""
    return ap.bitcast(F32R)


def _f(ap):
    """plain-fp32 view of an f32r AP (for DVE/ACT reads)."""
    return ap.bitcast(F32)


def build_nc():
    nc = bacc.Bacc(None, target_bir_lowering=False)
    t_xT = nc.dram_tensor("xT", [P, CT, T], F32R, kind="ExternalInput")
    t_qw = nc.dram_tensor("qw", [CT, P, CT, P], F32R, kind="ExternalInput")
    t_kw = nc.dram_tensor("kw", [KM, P, CT, P], F32R, kind="ExternalInput")
    t_vw = nc.dram_tensor("vw", [P, CT, N_KV * H], F32R, kind="ExternalInput")
    t_ow = nc.dram_tensor("ow", [CB, P, CT, 512], F32R, kind="ExternalInput")
    t_cq = nc.dram_tensor("cq", [P, TL], F32, kind="ExternalInput")
    t_sq = nc.dram_tensor("sq", [P, TL], F32, kind="ExternalInput")
    t_ck = nc.dram_tensor("ck", [P, T], F32, kind="ExternalInput")
    t_sk = nc.dram_tensor("sk", [P, T], F32, kind="ExternalInput")
    t_sw = nc.dram_tensor("sw", [P, P], F32R, kind="ExternalInput")
    t_on = nc.dram_tensor("on", [P, P], F32R, kind="ExternalInput")
    t_out = nc.dram_tensor("out", [TL, C], F32, kind="ExternalOutput")

    with tile.TileContext(nc) as tc:
        _emit(tc, t_xT, t_qw, t_kw, t_vw, t_ow,
              t_cq, t_sq, t_ck, t_sk, t_sw, t_on, t_out)
    nc.compile()
    return nc


def _rope(nc, dst, src, sw_ps, ctab, stab, tmp):
    """dst = src*ctab + sw_ps*stab.

    sw_ps is swap_halves(src) (PE permutation-matmul result in PSUM);
    the rotate_half sign lives in the stab table.
    """
    nc.vector.tensor_mul(dst, _f(src), ctab)
    nc.vector.tensor_mul(tmp, sw_ps, stab)
    nc.vector.tensor_add(dst, _f(dst), tmp)


def _emit(tc, t_xT, t_qw, t_kw, t_vw, t_ow, t_cq, t_sq, t_ck, t_sk,
          t_sw, t_on, t_out):
    nc = tc.nc

    with ExitStack() as ctx:
        persist = ctx.enter_context(tc.tile_pool(name="persist", bufs=1))
        kT = persist.tile([P, KM, T], F32R)
        ones = persist.tile([P, P], F32R)
        ones_b = persist.tile([P, P], BF16)
        nc.sync.dma_start(out=ones[:], in_=t_on[:])
        nc.vector.memset(ones_b[:], 1.0)
        # manual pools so early phases keep SBUF headroom for prefetch
        qTp = tc.alloc_tile_pool(name="qTp", bufs=1)
        vsbp = tc.alloc_tile_pool(name="vsbp", bufs=1)
        qT = None
        v_sb = None

        with ExitStack() as xctx:
            xtp = xctx.enter_context(tc.tile_pool(name="xt", bufs=1))
            xT = xtp.tile([P, CT, T], F32R)
            for ct in range(CT):
                nc.sync.dma_start(out=xT[:, ct, :], in_=t_xT[:, ct, :])

            # -------- K + Q projection / norm / rope (tables scoped) ------
            with ExitStack() as tctx:
                tabp = tctx.enter_context(tc.tile_pool(name="tab", bufs=1))
                sw = tabp.tile([P, P], F32R)
                eps_t = tabp.tile([P, 1], F32)
                ckr = tabp.tile([P, T], F32)
                skr = tabp.tile([P, T], F32)
                cqr = tabp.tile([P, TL], F32)
                sqr = tabp.tile([P, TL], F32)
                cq2 = tabp.tile([P, 2, TL], F32)
                sq2 = tabp.tile([P, 2, TL], F32)
                nc.vector.memset(eps_t[:], EPS)
                nc.sync.dma_start(out=sw[:], in_=t_sw[:])
                nc.sync.dma_start(out=ckr[:], in_=t_ck[:])
                nc.sync.dma_start(out=skr[:], in_=t_sk[:])
                nc.sync.dma_start(out=cqr[:], in_=t_cq[:])
                nc.sync.dma_start(out=sqr[:], in_=t_sq[:])

                # ------- Phase K (ct-outer: start on first x chunk) ----
                with ExitStack() as kctx:
                    wkp = kctx.enter_context(tc.tile_pool(name="wk", bufs=1))
                    krawp = kctx.enter_context(
                        tc.tile_pool(name="kraw", bufs=1))
                    ksqp = kctx.enter_context(
                        tc.tile_pool(name="ksq", bufs=KM))
                    tmpp = kctx.enter_context(
                        tc.tile_pool(name="ktmp", bufs=1))
                    rtmpp = kctx.enter_context(
                        tc.tile_pool(name="rtmp", bufs=2))

                    kraw = krawp.tile([P, KM, T], F32R)
                    wkt = wkp.tile([P, KM, CT, P], F32R)
                    for mt in range(KM):
                        nc.sync.dma_start(out=wkt[:, mt, :, :], in_=t_kw[mt])
                    ksqs = []
                    with ExitStack() as pctx:
                        ppk = pctx.enter_context(
                            tc.tile_pool(name="ppk", bufs=KM, space="PSUM"))
                        pks = []
                        for mt in range(KM):
                            pks.append(ppk.tile([P, T], F32, tag="pk", name="pk"))
                        for ct in range(CT):
                            for mt in range(KM):
                                for blk in range(2):
                                    nc.tensor.matmul(
                                        pks[mt][:,
                                                blk * 512:(blk + 1) * 512],
                                        _r(wkt[:, mt, ct, :]),
                                        _r(xT[:, ct,
                                              blk * 512:(blk + 1) * 512]),
                                        start=(ct == 0), stop=(ct == CT - 1))
                        for mt in range(KM):
                            ksq = ksqp.tile([P, T], F32R, tag="ksq")
                            nc.scalar.square(ksq[:], pks[mt][:])
                            nc.vector.tensor_copy(kraw[:, mt, :], pks[mt][:])
                            ksqs.append(ksq)
                    with ExitStack() as pctx:
                        pksum = pctx.enter_context(
                            tc.tile_pool(name="pks", bufs=1, space="PSUM"))
                        ksum = pksum.tile([P, T], F32)
                        for mt in range(KM):
                            for blk in range(2):
                                nc.tensor.matmul(
                                    ksum[:, blk * 512:(blk + 1) * 512],
                                    _r(ones[:]),
                                    _r(ksqs[mt][:,
                                                blk * 512:(blk + 1) * 512]),
                                    start=(mt == 0), stop=(mt == KM - 1))
                        srt = tmpp.tile([P, T], F32, tag="srt")
                        nc.scalar.activation(srt[:], ksum[:], AF.Sqrt,
                                             bias=eps_t[:],
                                             scale=1.0 / (N_KV * H))
                    rstd = tmpp.tile([P, T], F32, tag="rstd")
                    nc.vector.reciprocal_approx_fast(out=rstd[:], in_=srt[:])
                    nc.vector.tensor_mul(ckr[:], ckr[:], rstd[:])
                    nc.vector.tensor_mul(skr[:], skr[:], rstd[:])
                    with ExitStack() as pctx:
                        psw = pctx.enter_context(
                            tc.tile_pool(name="psw", bufs=KM, space="PSUM"))
                        ksws = []
                        for mt in range(KM):
                            ksw = psw.tile([P, T], F32, tag="ksw")
                            for blk in range(2):
                                nc.tensor.matmul(
                                    ksw[:, blk * 512:(blk + 1) * 512],
                                    _r(sw[:]),
                                    _r(kraw[:, mt,
                                            blk * 512:(blk + 1) * 512]),
                                    start=True, stop=True)
                            ksws.append(ksw)
                        for mt in range(KM):
                            rtmp = rtmpp.tile([P, T], F32, tag="rtmp")
                            _rope(nc, kT[:, mt, :], kraw[:, mt, :],
                                  ksws[mt][:], ckr[:], skr[:], rtmp[:])

                # ---------------- Phase Q ----------------
                with ExitStack() as qctx:
                    qT = qTp.tile([P, N_HEADS, TL], F32R)
                    wqp = qctx.enter_context(tc.tile_pool(name="wq", bufs=3))
                    qrawp = qctx.enter_context(
                        tc.tile_pool(name="qraw", bufs=1))
                    qsqp = qctx.enter_context(
                        tc.tile_pool(name="qsq", bufs=N_HEADS))
                    qtmpp = qctx.enter_context(
                        tc.tile_pool(name="qtmp", bufs=1))
                    qrtmpp = qctx.enter_context(
                        tc.tile_pool(name="qrtmp", bufs=2))

                    qraw = qrawp.tile([P, N_HEADS, TL], F32R)
                    qsqs = []
                    with ExitStack() as pctx:
                        ppq = pctx.enter_context(
                            tc.tile_pool(name="ppq", bufs=2, space="PSUM"))
                        pqs = pctx.enter_context(
                            tc.tile_pool(name="pqs", bufs=1, space="PSUM"))
                        for mt in range(N_HEADS):
                            wqt = wqp.tile([P, CT, P], F32R, tag="wq")
                            nc.sync.dma_start(out=wqt[:], in_=t_qw[mt])
                            pq = ppq.tile([P, TL], F32, tag="pq")
                            for ct in range(CT):
                                nc.tensor.matmul(
                                    pq[:], _r(wqt[:, ct, :]),
                                    _r(xT[:, ct, 0:TL]),
                                    start=(ct == 0), stop=(ct == CT - 1))
                            qsq = qsqp.tile([P, TL], F32R, tag="qsq")
                            nc.scalar.square(qsq[:], pq[:])
                            nc.vector.tensor_copy(qraw[:, mt, :], pq[:])
                            qsqs.append(qsq)
                        qsum = pqs.tile([P, TL], F32)
                        for mt in range(N_HEADS):
                            nc.tensor.matmul(qsum[:], _r(ones[:]),
                                             _r(qsqs[mt][:]),
                                             start=(mt == 0),
                                             stop=(mt == N_HEADS - 1))
                        srtq = qtmpp.tile([P, TL], F32, tag="srtq")
                        nc.scalar.activation(srtq[:], qsum[:], AF.Sqrt,
                                             bias=eps_t[:],
                                             scale=1.0 / (N_HEADS * H))
                    rstdq = qtmpp.tile([P, TL], F32, tag="rstdq")
                    nc.vector.reciprocal_approx_fast(out=rstdq[:],
                                                     in_=srtq[:])
                    for h in range(2):
                        nc.vector.tensor_mul(cq2[:, h, :], cqr[:], rstdq[:])
                        nc.vector.tensor_mul(sq2[:, h, :], sqr[:], rstdq[:])
                    with ExitStack() as pctx:
                        pswq = pctx.enter_context(
                            tc.tile_pool(name="pswq", bufs=N_HEADS // 2,
                                         space="PSUM"))
                        qsws = []
                        for j in range(N_HEADS // 2):
                            qsw = pswq.tile([P, 2, TL], F32, tag="qsw")
                            for h in range(2):
                                nc.tensor.matmul(qsw[:, h, :], _r(sw[:]),
                                                 _r(qraw[:, 2 * j + h, :]),
                                                 start=True, stop=True)
                            qsws.append(qsw)
                        # rope per pair: full-tile PSUM reads only after
                        # both halves are written (PSUM bank safety)
                        for j in range(N_HEADS // 2):
                            qtmp = qrtmpp.tile([P, 2, TL], F32, tag="qrtmp")
                            _rope(nc, qT[:, 2 * j:2 * j + 2, :],
                                  qraw[:, 2 * j:2 * j + 2, :],
                                  qsws[j][:], cq2[:], sq2[:], qtmp[:])

            # ---------------- Phase V ----------------
            with ExitStack() as vctx:
                v_sb = vsbp.tile([P, ST, N_KV * H], BF16)
                vwp = vctx.enter_context(tc.tile_pool(name="vw", bufs=1))
                ppv = vctx.enter_context(
                    tc.tile_pool(name="ppv", bufs=2, space="PSUM"))
                vw = vwp.tile([P, CT, N_KV * H], F32R)
                for c4 in range(4):
                    nc.sync.dma_start(out=vw[:, 4 * c4:4 * c4 + 4, :],
                                      in_=t_vw[:, 4 * c4:4 * c4 + 4, :])
                for tt in range(ST):
                    pv = ppv.tile([P, N_KV * H], F32, tag="pv")
                    for ct in range(CT):
                        nc.tensor.matmul(
                            pv[:], _r(xT[:, ct, tt * P:(tt + 1) * P]),
                            _r(vw[:, ct, :]),
                            start=(ct == 0), stop=(ct == CT - 1))
                    nc.vector.tensor_copy(v_sb[:, tt, :], pv[:])

        # ---------------- Phase A: attention ------------------------------
        with ExitStack() as actx:
            attnp = actx.enter_context(tc.tile_pool(name="attn", bufs=1))
            encT = attnp.tile([P, N_HEADS, TL], F32R)
            owp = actx.enter_context(tc.tile_pool(name="ow", bufs=3))
            # prefetch out-proj weight blocks during attention
            owts = []
            for cb in range(3):
                owt = owp.tile([P, CT, 512], F32R, tag="ow")
                nc.sync.dma_start(out=owt[:], in_=t_ow[cb])
                owts.append(owt)

            with ExitStack() as kvctx:
                expp = kvctx.enter_context(tc.tile_pool(name="exp", bufs=1))
                rcpp = kvctx.enter_context(tc.tile_pool(name="rcp", bufs=2))
                lp = kvctx.enter_context(
                    tc.tile_pool(name="lp", bufs=3, space="PSUM"))
                sp = kvctx.enter_context(
                    tc.tile_pool(name="sp", bufs=1, space="PSUM"))
                ep = kvctx.enter_context(
                    tc.tile_pool(name="ep", bufs=1, space="PSUM"))

                for kh in range(N_KV):
                    ex = expp.tile([P, ST, G, TL], BF16, tag="ex")
                    for pair in range(2):
                        hlo = 2 * pair
                        q_rhs = qT[:, G * kh + hlo:G * kh + hlo + 2, :]
                        for st2 in range(ST // 2):
                            L = lp.tile([P, 2, 2, TL], F32, tag="L")
                            for j in range(2):
                                st = st2 * 2 + j
                                nc.tensor.matmul(
                                    L[:, j, :, :],
                                    _r(kT[:, kh, st * P:(st + 1) * P]),
                                    _r(q_rhs), start=True, stop=True)
                            nc.scalar.activation(
                                ex[:, st2 * 2:st2 * 2 + 2, hlo:hlo + 2, :],
                                L[:], AF.Exp)
                        # softmax denominators, replicated over partitions
                        S = sp.tile([P, 2, TL], F32, tag="S")
                        for st in range(ST):
                            nc.tensor.matmul(
                                S[:], ones_b[:],
                                ex[:, st, hlo:hlo + 2, :],
                                start=(st == 0), stop=(st == ST - 1))
                        # probs @ v  (unnormalized)
                        E = ep.tile([P, 2, TL], F32, tag="E")
                        for st in range(ST):
                            nc.tensor.matmul(
                                E[:], v_sb[:, st, kh * H:(kh + 1) * H],
                                ex[:, st, hlo:hlo + 2, :],
                                start=(st == 0), stop=(st == ST - 1))
                        # normalize while draining PSUM -> SBUF
                        rcp = rcpp.tile([P, 2, TL], F32, tag="rcp")
                        nc.vector.reciprocal_approx_fast(out=rcp[:],
                                                         in_=S[:])
                        nc.vector.tensor_mul(
                            encT[:, G * kh + hlo:G * kh + hlo + 2, :],
                            E[:], rcp[:])

            # ---------------- Phase O: output projection ------------------
            with ExitStack() as octx:
                otp = octx.enter_context(tc.tile_pool(name="ot", bufs=3))
                pop = octx.enter_context(
                    tc.tile_pool(name="po", bufs=2, space="PSUM"))
                for cb in range(CB):
                    if cb < 3:
                        owt = owts[cb]
                    else:
                        owt = owp.tile([P, CT, 512], F32R, tag="ow")
                        nc.sync.dma_start(out=owt[:], in_=t_ow[cb])
                    for tt in range(TL // P):
                        PO = pop.tile([P, 512], F32, tag="PO")
                        for mt in range(CT):
                            nc.tensor.matmul(
                                PO[:], _r(encT[:, mt, tt * P:(tt + 1) * P]),
                                _r(owt[:, mt, :]),
                                start=(mt == 0), stop=(mt == CT - 1))
                        o = otp.tile([P, 512], F32, tag="o")
                        nc.vector.tensor_copy(o[:], PO[:])
                        nc.sync.dma_start(
                            out=t_out[tt * P:(tt + 1) * P,
                                      cb * 512:(cb + 1) * 512],
                            in_=o[:])
        vsbp.release()
        qTp.release()


# ---------------------------------------------------------------------------
# host side: input prep, sharding, gather
# ---------------------------------------------------------------------------

def _tables():
    fraction = np.arange(0, H, 2, dtype=np.float32) / np.float32(H)
    inv_freq = (1.0 / (MAX_TIMESCALE ** fraction)).astype(np.float32)
    sinusoid = np.arange(T, dtype=np.float32)[:, None] * inv_freq[None, :]
    sinusoid = np.concatenate([sinusoid, sinusoid], axis=-1)  # [T, H]
    sinT = np.sin(sinusoid).T.astype(np.float32)              # [H, T]
    cosT = np.cos(sinusoid).T.astype(np.float32)
    sin_signed = np.concatenate([-sinT[:H // 2], sinT[H // 2:]], axis=0)
    scale = np.float32(1.0) / np.sqrt(np.float32(H)).astype(np.float32)
    return (cosT.copy(), sin_signed.copy(),
            (cosT * scale).astype(np.float32),
            (sin_signed * scale).astype(np.float32))


def make_in_maps(x, q_kernel, k_kernel, v_kernel, out_kernel):
    x = np.ascontiguousarray(np.asarray(x, dtype=np.float32))
    qk = np.asarray(q_kernel, dtype=np.float32)
    kk = np.asarray(k_kernel, dtype=np.float32)
    vk = np.asarray(v_kernel, dtype=np.float32)
    ok = np.asarray(out_kernel, dtype=np.float32)

    qw = np.ascontiguousarray(qk.reshape(CT, P, CT, P).transpose(2, 1, 0, 3))
    kw = np.ascontiguousarray(kk.reshape(CT, P, KM, P).transpose(2, 1, 0, 3))
    vw = np.ascontiguousarray(vk.reshape(CT, P, N_KV * H).transpose(1, 0, 2))
    ow = np.ascontiguousarray(
        ok.reshape(CT, P, CB, 512).transpose(2, 1, 0, 3))
    ck_h, sk_h, cq_full, sq_full = _tables()
    sw_h = np.zeros((P, P), np.float32)
    sw_h[(np.arange(P) + P // 2) % P, np.arange(P)] = 1.0
    on_h = np.ones((P, P), np.float32)

    xt = [np.ascontiguousarray(
        x[b].T.reshape(CT, P, T).transpose(1, 0, 2)) for b in range(B)]

    in_maps = []
    for core in range(8):
        b, s = divmod(core, 4)
        t0 = s * TL
        # roll the key/value sequence so this core's query block is first;
        # softmax over s is permutation-invariant, RoPE tables roll along.
        in_maps.append({
            "xT": np.ascontiguousarray(np.roll(xt[b], -t0, axis=2)),
            "qw": qw, "kw": kw, "vw": vw, "ow": ow,
            "cq": np.ascontiguousarray(
                np.roll(cq_full, -t0, axis=1)[:, :TL]),
            "sq": np.ascontiguousarray(
                np.roll(sq_full, -t0, axis=1)[:, :TL]),
            "ck": np.ascontiguousarray(np.roll(ck_h, -t0, axis=1)),
            "sk": np.ascontiguousarray(np.roll(sk_h, -t0, axis=1)),
            "sw": sw_h, "on": on_h,
        })
    return in_maps


def _install_trace_shim():
    """Dev-only (KERNEL_TRACE=1): register the NTFF profile hook that this
    agent image's antenv lacks, and skip the artifact cloud upload."""
    import sys
    import types
    try:
        from antenv import axon_hooks  # noqa: F401
        ok = True
    except ImportError:
        try:
            from trn_agent_boot.trn_boot import _ntff_profile_via_ctypes
            hook = _ntff_profile_via_ctypes("/opt/axon/libaxon_pjrt.so")
            m = types.ModuleType("antenv.axon_hooks")
            m.get_axon_ntff_profile_hook = lambda: hook
            m.set_axon_ntff_profile_hook = lambda h: None
            sys.modules["antenv.axon_hooks"] = m
            ok = True
        except Exception as e:  # profiling unavailable; still run
            print(f"trace shim failed: {e!r}")
            ok = False
    if ok:
        import concourse.bass_utils as bu
        bu.upload_artifacts = lambda tmpdir: tmpdir
    return ok


def kernel(x, q_kernel, k_kernel, v_kernel, out_kernel):
    in_maps = make_in_maps(x, q_kernel, k_kernel, v_kernel, out_kernel)
    nc = build_nc()
    trace = bool(os.environ.get("KERNEL_TRACE"))
    kwargs = {}
    if trace:
        trace = _install_trace_shim()
        if trace:
            tdir = os.environ.get("KERNEL_TRACE_DIR")
            if tdir:
                os.makedirs(tdir, exist_ok=True)
                kwargs["tmpdir"] = tdir
    res = run_bass_kernel_spmd(nc, in_maps, core_ids=list(range(8)),
                               trace=trace, **kwargs)
    out = np.zeros((B, T, C), np.float32)
    for core in range(8):
        b, s = divmod(core, 4)
        out[b, s * TL:(s + 1) * TL] = res.results[core]["out"]
    if trace:
        kernel.last_exec_time_ns = res.exec_time_ns
        kernel.last_profile = res.profile_json
    return out
